# revision 1
# baseline (speedup 1.0000x reference)
"""v3 config snapshot (measured 170.6us once): 5-tap taper groups,
greedy queue balance (which happened to alternate rings), single xtile
split across both HWDGE queue heads, kpool bufs=3, tmp bufs=5."""

import numpy as np

B, C, H, W, K = 4, 32, 256, 256, 5
P = (K - 1) // 2
CP = 16
YG = 8
RG = H // YG
WP = W + 2 * P
SROWS = RG + 2 * P
SLEN = SROWS * WP
HR = RG // 2
XROWS = HR + 2 * P  # 20 rows per half-stripe (halo incl)
XLEN = XROWS * WP   # 5200 elems per partition half-stripe
HFREE = HR * W
HHALF = HFREE // 2
NBANK = HFREE // 512
GMAX = 5
GFREE = GMAX * HFREE

GROUPS = [
    (0, 0, 1), (0, 1, 2), (0, 3, 3), (0, 6, 4), (0, 10, 5), (0, 15, 5),
    (0, 20, 5),
    (1, 0, 5), (1, 5, 5), (1, 10, 5), (1, 15, 5), (1, 20, 2), (1, 22, 2),
    (1, 24, 1),
]
KTOTAL = 128 * 2 * K * K * HFREE

_cache = {}


def _build_nc():
    import concourse.bass as bass
    import concourse.tile as tile
    from concourse import bacc, mybir

    f32 = mybir.dt.float32
    f16 = mybir.dt.float16
    nc = bacc.Bacc("TRN2", target_bir_lowering=False, debug=False, num_devices=8)

    xs_t = nc.dram_tensor("xs", [128, 2 * XLEN], f16, kind="ExternalInput")
    ks_t = nc.dram_tensor("ks", [KTOTAL], f16, kind="ExternalInput")
    ident_t = nc.dram_tensor("ident", [128, 128], f16, kind="ExternalInput")
    out_t = nc.dram_tensor("out", [128, 2 * HFREE], f16, kind="ExternalOutput")

    with tile.TileContext(nc) as tc:
        with (
            tc.tile_pool(name="xp", bufs=1) as xpool,
            tc.tile_pool(name="idp", bufs=1) as ipool,
            tc.tile_pool(name="kp", bufs=3) as kpool,
            tc.tile_pool(name="tp", bufs=5) as tpool,
            tc.tile_pool(name="op", bufs=2) as opool,
            tc.tile_pool(name="pp", bufs=1, space="PSUM") as ppool,
        ):
            # Only the h=0 x rows head the two rings (1.33MB, smallest
            # blocker for the first tap product); the h=1 rows are
            # issued mid-stream on a fast HWDGE ring just before the
            # h=1 kernel groups (same total bytes, earlier pipeline
            # start, and no slow SWDGE interference).
            ident = ipool.tile([128, 128], f16)
            nc.gpsimd.dma_start(out=ident[:], in_=ident_t[:, :])

            xtA = xpool.tile([128, XLEN], f16, tag="xa")
            xtB = xpool.tile([128, XLEN], f16, tag="xb")
            XH = XLEN // 2
            nc.sync.dma_start(out=xtA[:, :XH], in_=xs_t[:, :XH])
            nc.scalar.dma_start(out=xtA[:, XH:], in_=xs_t[:, XH:XLEN])

            x3h = [
                xtA[:].rearrange("p (r w) -> p r w", w=WP),
                xtB[:].rearrange("p (r w) -> p r w", w=WP),
            ]

            qbytes = {"sync": XLEN // 2, "scalar": XLEN // 2}
            off = 0
            for h, t0, nt in GROUPS:
                if h == 1 and t0 == 0:
                    qname = min(qbytes, key=qbytes.get)
                    qbytes[qname] += XLEN
                    xeng = nc.sync if qname == "sync" else nc.scalar
                    xeng.dma_start(out=xtB[:], in_=xs_t[:, XLEN:])
                glen = nt * HFREE
                ktile = kpool.tile([128, GFREE], f16, tag="kt")
                ksrc = bass.AP(ks_t, off, [[glen, 128], [1, glen]])
                off += 128 * glen
                qname = min(qbytes, key=qbytes.get)
                qbytes[qname] += glen
                keng = nc.sync if qname == "sync" else nc.scalar
                keng.dma_start(out=ktile[:, :glen], in_=ksrc)

                if t0 == 0:
                    ptile = ppool.tile([128, HFREE], f32, tag="ps")
                for t in range(nt):
                    ij = t0 + t
                    i, j = divmod(ij, K)
                    k3 = ktile[:, t * HFREE : (t + 1) * HFREE].rearrange(
                        "p (r w) -> p r w", w=W
                    )
                    xv = x3h[h][:, i : i + HR, j : j + W]
                    tmp = tpool.tile([128, HFREE], f16, tag="tmp")
                    t3 = tmp[:].rearrange("p (r w) -> p r w", w=W)
                    nc.vector.tensor_mul(t3, xv, k3)
                    for bk in range(NBANK):
                        nc.tensor.matmul(
                            out=ptile[:, bk * 512 : (bk + 1) * 512],
                            lhsT=ident[:],
                            rhs=tmp[:, bk * 512 : (bk + 1) * 512],
                            start=(ij == 0),
                            stop=(ij == K * K - 1),
                        )

                if t0 + nt == K * K:
                    for q in range(2):
                        ob = opool.tile([128, HHALF], f16, tag="ob")
                        src = ptile[:, q * HHALF : (q + 1) * HHALF]
                        if h == 1 and q == 1:
                            nc.vector.tensor_copy(ob[:], src)
                        else:
                            nc.scalar.copy(ob[:], src)
                        dst = bass.AP(
                            out_t,
                            h * HFREE + q * HHALF,
                            [[2 * HFREE, 128], [1, HHALF]],
                        )
                        if h == 0:
                            nc.gpsimd.dma_start(out=dst, in_=ob[:])
                        else:
                            seng = nc.sync if q == 0 else nc.scalar
                            seng.dma_start(out=dst, in_=ob[:])

    nc.compile()
    return nc


def _get_nc():
    if "nc" not in _cache:
        _cache["nc"] = _build_nc()
    return _cache["nc"]


_IDENT = np.eye(128, dtype=np.float16)
# part A = stripe rows 0..19 (h=0), part B = stripe rows 16..35 (h=1)
_ROWIDXA = (np.arange(YG)[:, None] * RG + np.arange(XROWS)[None, :])
_ROWIDXB = _ROWIDXA + HR


def _make_in_maps(x, kernel):
    x = np.asarray(x, dtype=np.float32).astype(np.float16)
    kern = np.asarray(kernel, dtype=np.float32).astype(np.float16)
    xpad = np.pad(x, ((0, 0), (0, 0), (P, P), (P, P)), mode="edge")

    in_maps = []
    for core in range(8):
        b, half = divmod(core, 2)
        c0 = half * CP
        xp_c = xpad[b, c0 : c0 + CP]
        xs = np.concatenate(
            [
                xp_c[:, _ROWIDXA, :].reshape(128, XLEN),
                xp_c[:, _ROWIDXB, :].reshape(128, XLEN),
            ],
            axis=1,
        )
        kc = kern[b, c0 * K * K : (c0 + CP) * K * K]
        kc = kc.reshape(CP, K * K, YG, 2, HR, W).transpose(3, 1, 0, 2, 4, 5)
        ks = np.empty(KTOTAL, dtype=np.float16)
        off = 0
        for h, t0, nt in GROUPS:
            reg = kc[h, t0 : t0 + nt].transpose(1, 2, 0, 3, 4)
            n = 128 * nt * HFREE
            ks[off : off + n] = reg.reshape(-1)
            off += n
        in_maps.append(
            {"xs": np.ascontiguousarray(xs), "ks": ks, "ident": _IDENT}
        )
    return in_maps


def kernel(x, kernel, kernel_size):
    from concourse.bass_utils import run_bass_kernel_spmd

    in_maps = _make_in_maps(x, kernel)
    nc = _get_nc()
    res = run_bass_kernel_spmd(nc, in_maps, list(range(8)))

    out = np.empty((B, C, H, W), dtype=np.float32)
    for core in range(8):
        b, half = divmod(core, 2)
        c0 = half * CP
        o = res.results[core]["out"].reshape(CP, YG, 2, HR, W)
        out[b, c0 : c0 + CP] = o.reshape(CP, H, W).astype(np.float32)
    return out



# revision 2
# speedup vs baseline: 1.0052x; 1.0052x over previous
"""v5 (best measured): fp8(e3m4) A-units upcast on ACT + f16 F-units,
all muls single-tap on DVE (2x mode). PE identity-matmul accumulate.
a=29 fp8 units, f=21 f16 units -> DMA ~43MB/core (vs 57 all-f16).
"""

import numpy as np
import ml_dtypes

B, C, H, W, K = 4, 32, 256, 256, 5
P = (K - 1) // 2
CP = 16
YG = 8
RG = H // YG
HR = RG // 2
WP = W + 2 * P
XROWS = HR + 2 * P
XLEN = XROWS * WP  # 5200
XLEN2 = XLEN + 4
HFREE = HR * W  # 4096
HHALF = HFREE // 2
NBANK = HFREE // 512
UF = HFREE

FSLOTS = {
    0: (0, 1, 3, 4, 8, 9, 15, 16, 23, 24),
    1: (0, 1, 3, 4, 8, 9, 15, 16, 19, 23, 24),
}


def _cats(h):
    return {s: ("F" if s in FSLOTS[h] else "A") for s in range(25)}


def _batches(h, cat):
    out = {}
    for st in "FA":
        slots = [s for s in range(25) if cat[s] == st]
        bs = []
        if st == "F" and h == 0:
            bs.append([slots[0]])
            slots = slots[1:]
        while slots:
            bs.append(slots[:2])
            slots = slots[2:]
        out[st] = bs
    return out


_SCHED = []
for h in (0, 1):
    cat = _cats(h)
    _SCHED.append({"cat": cat, "batches": _batches(h, cat)})

_cache = {}


def _build_nc():
    import concourse.bass as bass
    import concourse.tile as tile
    from concourse import bacc, mybir

    f32 = mybir.dt.float32
    f16 = mybir.dt.float16
    f8 = mybir.dt.float8e3
    nc = bacc.Bacc("TRN2", target_bir_lowering=False, debug=False, num_devices=8)

    nA = sum(1 for h in (0, 1) for s in range(25) if _SCHED[h]["cat"][s] == "A")
    nF = sum(1 for h in (0, 1) for s in range(25) if _SCHED[h]["cat"][s] == "F")

    xs_t = nc.dram_tensor("xs", [128, 2 * XLEN2], f16, kind="ExternalInput")
    kA_t = nc.dram_tensor("kA", [nA * 128 * UF], f8, kind="ExternalInput")
    kF_t = nc.dram_tensor("kF", [nF * 128 * UF], f16, kind="ExternalInput")
    ident_t = nc.dram_tensor("ident", [128, 128], f16, kind="ExternalInput")
    out_t = nc.dram_tensor("out", [128, 2 * HFREE], f16, kind="ExternalOutput")

    with tile.TileContext(nc) as tc:
        with (
            tc.tile_pool(name="xp", bufs=1) as xpool,
            tc.tile_pool(name="a8", bufs=3) as a8pool,
            tc.tile_pool(name="a16", bufs=6) as a16pool,
            tc.tile_pool(name="f16s", bufs=2) as fpool,
            tc.tile_pool(name="tp", bufs=5) as tpool,
            tc.tile_pool(name="op", bufs=4) as opool,
            tc.tile_pool(name="pp", bufs=1, space="PSUM") as ppool,
        ):
            ident = xpool.tile([128, 128], f16)
            nc.scalar.dma_start(out=ident[:], in_=ident_t[:, :])
            xt = xpool.tile([128, 2 * XLEN2], f16)
            nc.scalar.dma_start(out=xt[:, :XLEN2], in_=xs_t[:, :XLEN2])
            nc.scalar.dma_start(out=xt[:, XLEN2:], in_=xs_t[:, XLEN2:])

            x3h = [
                xt[:, h * XLEN2 : h * XLEN2 + XLEN].rearrange(
                    "p (r w) -> p r w", w=WP
                )
                for h in (0, 1)
            ]

            offs = {"A": 0, "F": 0}
            ktensor = {"A": kA_t, "F": kF_t}
            kdt = {"A": f8, "F": f16}
            kpool = {"A": a8pool, "F": fpool}
            ktag = {"A": "a8", "F": "f16"}

            pending_stores = []

            for h in (0, 1):
                cat = _SCHED[h]["cat"]
                batches = _SCHED[h]["batches"]
                first_slot = {st: {b[0]: b for b in bs} for st, bs in batches.items()}
                unit_view = {}

                ptile = ppool.tile([128, HFREE], f32, tag="ps")

                for slot in range(25):
                    st = cat[slot]
                    if h == 1 and slot in (2, 5) and pending_stores:
                        dst, ob = pending_stores.pop(0)
                        nc.gpsimd.dma_start(out=dst, in_=ob[:])

                    if slot in first_slot[st]:
                        bslots = first_slot[st][slot]
                        n = len(bslots)
                        ktile = kpool[st].tile([128, 2 * UF], kdt[st], tag=ktag[st])
                        src = bass.AP(
                            ktensor[st], offs[st], [[n * UF, 128], [1, n * UF]]
                        )
                        offs[st] += 128 * n * UF
                        nc.sync.dma_start(out=ktile[:, : n * UF], in_=src)
                        for q, s2 in enumerate(bslots):
                            unit_view[s2] = ktile[:, q * UF : (q + 1) * UF]

                    i, j = divmod(slot, K)
                    if st == "A":
                        k16 = a16pool.tile([128, UF], f16, tag="a16")
                        nc.scalar.copy(k16[:], unit_view[slot])
                        ksrc = k16[:]
                    else:
                        ksrc = unit_view[slot]
                    tmp = tpool.tile([128, UF], f16, tag="tmp")
                    t3 = tmp[:].rearrange("p (r w) -> p r w", w=W)
                    k3 = ksrc.rearrange("p (r w) -> p r w", w=W)
                    xv = x3h[h][:, i : i + HR, j : j + W]
                    nc.vector.tensor_mul(t3, xv, k3)
                    for bk in range(NBANK):
                        nc.tensor.matmul(
                            out=ptile[:, bk * 512 : (bk + 1) * 512],
                            lhsT=ident[:],
                            rhs=tmp[:, bk * 512 : (bk + 1) * 512],
                            start=(slot == 0),
                            stop=(slot == 24),
                        )

                for q in range(2):
                    ob = opool.tile([128, HHALF], f16, tag="ob")
                    nc.scalar.copy(ob[:], ptile[:, q * HHALF : (q + 1) * HHALF])
                    dst = bass.AP(
                        out_t,
                        h * HFREE + q * HHALF,
                        [[2 * HFREE, 128], [1, HHALF]],
                    )
                    if h == 0:
                        pending_stores.append((dst, ob))
                    else:
                        seng = nc.sync if q == 0 else nc.scalar
                        seng.dma_start(out=dst, in_=ob[:])

    nc.compile()
    return nc


def _get_nc():
    if "nc" not in _cache:
        _cache["nc"] = _build_nc()
    return _cache["nc"]


_IDENT = np.eye(128, dtype=np.float16)
_ROWIDXA = np.arange(YG)[:, None] * RG + np.arange(XROWS)[None, :]
_ROWIDXB = _ROWIDXA + HR


def _make_in_maps(x, kernel):
    x = np.asarray(x, dtype=np.float32).astype(np.float16)
    kern = np.asarray(kernel, dtype=np.float32)
    xpad = np.pad(x, ((0, 0), (0, 0), (P, P), (P, P)), mode="edge")

    in_maps = []
    for core in range(8):
        b, half = divmod(core, 2)
        c0 = half * CP
        xp_c = xpad[b, c0 : c0 + CP]
        xs = np.zeros((128, 2 * XLEN2), dtype=np.float16)
        xs[:, :XLEN] = xp_c[:, _ROWIDXA, :].reshape(128, XLEN)
        xs[:, XLEN2 : XLEN2 + XLEN] = xp_c[:, _ROWIDXB, :].reshape(128, XLEN)

        kk = kern[b, c0 * K * K : (c0 + CP) * K * K].reshape(
            CP, K * K, YG, 2, HR, W
        )

        kA_blocks, kF_blocks = [], []
        for h in (0, 1):
            for st, dstl in (("A", kA_blocks), ("F", kF_blocks)):
                for bs in _SCHED[h]["batches"][st]:
                    blks = []
                    for s in bs:
                        blk = kk[:, s, :, h].reshape(128, UF)
                        if st == "A":
                            blks.append(
                                np.clip(blk, -15.5, 15.5).astype(
                                    ml_dtypes.float8_e3m4
                                )
                            )
                        else:
                            blks.append(blk.astype(np.float16))
                    dstl.append(np.concatenate(blks, axis=1))

        in_maps.append(
            {
                "xs": np.ascontiguousarray(xs),
                "kA": np.concatenate([b_.reshape(-1) for b_ in kA_blocks]),
                "kF": np.concatenate([b_.reshape(-1) for b_ in kF_blocks]),
                "ident": _IDENT,
            }
        )
    return in_maps


def kernel(x, kernel, kernel_size):
    from concourse.bass_utils import run_bass_kernel_spmd

    in_maps = _make_in_maps(x, kernel)
    nc = _get_nc()
    res = run_bass_kernel_spmd(nc, in_maps, list(range(8)))

    out = np.empty((B, C, H, W), dtype=np.float32)
    for core in range(8):
        b, half = divmod(core, 2)
        c0 = half * CP
        o = res.results[core]["out"].reshape(CP, YG, 2, HR, W)
        out[b, c0 : c0 + CP] = o.reshape(CP, H, W).astype(np.float32)
    return out


# revision 3
# speedup vs baseline: 1.0087x; 1.0035x over previous
"""v5 (best measured): fp8(e3m4) A-units upcast on ACT + f16 F-units,
all muls single-tap on DVE (2x mode). PE identity-matmul accumulate.
a=29 fp8 units, f=21 f16 units -> DMA ~43MB/core (vs 57 all-f16).
"""

import numpy as np
import ml_dtypes

B, C, H, W, K = 4, 32, 256, 256, 5
P = (K - 1) // 2
CP = 16
YG = 8
RG = H // YG
HR = RG // 2
WP = W + 2 * P
XROWS = HR + 2 * P
XLEN = XROWS * WP  # 5200
XLEN2 = XLEN + 4
HFREE = HR * W  # 4096
HHALF = HFREE // 2
NBANK = HFREE // 512
UF = HFREE

FSLOTS = {
    0: (0, 1, 4, 7, 10, 13, 16, 19, 22, 24),
    1: (0, 1, 2, 4, 7, 10, 13, 16, 19, 22, 24),
}


def _cats(h):
    return {s: ("F" if s in FSLOTS[h] else "A") for s in range(25)}


def _batches(h, cat):
    out = {}
    for st in "FA":
        slots = [s for s in range(25) if cat[s] == st]
        bs = []
        if st == "F" and h == 0:
            bs.append([slots[0]])
            slots = slots[1:]
        while slots:
            bs.append(slots[:2])
            slots = slots[2:]
        out[st] = bs
    return out


_SCHED = []
for h in (0, 1):
    cat = _cats(h)
    _SCHED.append({"cat": cat, "batches": _batches(h, cat)})

_cache = {}


def _build_nc():
    import concourse.bass as bass
    import concourse.tile as tile
    from concourse import bacc, mybir

    f32 = mybir.dt.float32
    f16 = mybir.dt.float16
    f8 = mybir.dt.float8e3
    nc = bacc.Bacc("TRN2", target_bir_lowering=False, debug=False, num_devices=8)

    nA = sum(1 for h in (0, 1) for s in range(25) if _SCHED[h]["cat"][s] == "A")
    nF = sum(1 for h in (0, 1) for s in range(25) if _SCHED[h]["cat"][s] == "F")

    xs_t = nc.dram_tensor("xs", [128, 2 * XLEN2], f16, kind="ExternalInput")
    kA_t = nc.dram_tensor("kA", [nA * 128 * UF], f8, kind="ExternalInput")
    kF_t = nc.dram_tensor("kF", [nF * 128 * UF], f16, kind="ExternalInput")
    ident_t = nc.dram_tensor("ident", [128, 128], f16, kind="ExternalInput")
    out_t = nc.dram_tensor("out", [128, 2 * HFREE], f16, kind="ExternalOutput")

    with tile.TileContext(nc) as tc:
        with (
            tc.tile_pool(name="xp", bufs=1) as xpool,
            tc.tile_pool(name="a8", bufs=3) as a8pool,
            tc.tile_pool(name="a16", bufs=6) as a16pool,
            tc.tile_pool(name="f16s", bufs=2) as fpool,
            tc.tile_pool(name="tp", bufs=5) as tpool,
            tc.tile_pool(name="op", bufs=4) as opool,
            tc.tile_pool(name="pp", bufs=1, space="PSUM") as ppool,
        ):
            ident = xpool.tile([128, 128], f16)
            nc.scalar.dma_start(out=ident[:], in_=ident_t[:, :])
            xt = xpool.tile([128, 2 * XLEN2], f16)
            nc.scalar.dma_start(out=xt[:, :XLEN2], in_=xs_t[:, :XLEN2])
            nc.scalar.dma_start(out=xt[:, XLEN2:], in_=xs_t[:, XLEN2:])

            x3h = [
                xt[:, h * XLEN2 : h * XLEN2 + XLEN].rearrange(
                    "p (r w) -> p r w", w=WP
                )
                for h in (0, 1)
            ]

            offs = {"A": 0, "F": 0}
            ktensor = {"A": kA_t, "F": kF_t}
            kdt = {"A": f8, "F": f16}
            kpool = {"A": a8pool, "F": fpool}
            ktag = {"A": "a8", "F": "f16"}

            pending_stores = []

            for h in (0, 1):
                cat = _SCHED[h]["cat"]
                batches = _SCHED[h]["batches"]
                first_slot = {st: {b[0]: b for b in bs} for st, bs in batches.items()}
                unit_view = {}

                ptile = ppool.tile([128, HFREE], f32, tag="ps")

                for slot in range(25):
                    st = cat[slot]
                    if h == 1 and slot in (2, 5) and pending_stores:
                        dst, ob = pending_stores.pop(0)
                        nc.gpsimd.dma_start(out=dst, in_=ob[:])

                    if slot in first_slot[st]:
                        bslots = first_slot[st][slot]
                        n = len(bslots)
                        ktile = kpool[st].tile([128, 2 * UF], kdt[st], tag=ktag[st])
                        src = bass.AP(
                            ktensor[st], offs[st], [[n * UF, 128], [1, n * UF]]
                        )
                        offs[st] += 128 * n * UF
                        nc.sync.dma_start(out=ktile[:, : n * UF], in_=src)
                        for q, s2 in enumerate(bslots):
                            unit_view[s2] = ktile[:, q * UF : (q + 1) * UF]

                    i, j = divmod(slot, K)
                    if st == "A":
                        k16 = a16pool.tile([128, UF], f16, tag="a16")
                        nc.scalar.copy(k16[:], unit_view[slot])
                        ksrc = k16[:]
                    else:
                        ksrc = unit_view[slot]
                    tmp = tpool.tile([128, UF], f16, tag="tmp")
                    t3 = tmp[:].rearrange("p (r w) -> p r w", w=W)
                    k3 = ksrc.rearrange("p (r w) -> p r w", w=W)
                    xv = x3h[h][:, i : i + HR, j : j + W]
                    nc.vector.tensor_mul(t3, xv, k3)
                    for bk in range(NBANK):
                        nc.tensor.matmul(
                            out=ptile[:, bk * 512 : (bk + 1) * 512],
                            lhsT=ident[:],
                            rhs=tmp[:, bk * 512 : (bk + 1) * 512],
                            start=(slot == 0),
                            stop=(slot == 24),
                        )

                for q in range(2):
                    ob = opool.tile([128, HHALF], f16, tag="ob")
                    nc.scalar.copy(ob[:], ptile[:, q * HHALF : (q + 1) * HHALF])
                    dst = bass.AP(
                        out_t,
                        h * HFREE + q * HHALF,
                        [[2 * HFREE, 128], [1, HHALF]],
                    )
                    if h == 0:
                        pending_stores.append((dst, ob))
                    else:
                        seng = nc.sync if q == 0 else nc.scalar
                        seng.dma_start(out=dst, in_=ob[:])

    nc.compile()
    return nc


def _get_nc():
    if "nc" not in _cache:
        _cache["nc"] = _build_nc()
    return _cache["nc"]


_IDENT = np.eye(128, dtype=np.float16)
_ROWIDXA = np.arange(YG)[:, None] * RG + np.arange(XROWS)[None, :]
_ROWIDXB = _ROWIDXA + HR


def _make_in_maps(x, kernel):
    x = np.asarray(x, dtype=np.float32).astype(np.float16)
    kern = np.asarray(kernel, dtype=np.float32)
    xpad = np.pad(x, ((0, 0), (0, 0), (P, P), (P, P)), mode="edge")

    in_maps = []
    for core in range(8):
        b, half = divmod(core, 2)
        c0 = half * CP
        xp_c = xpad[b, c0 : c0 + CP]
        xs = np.zeros((128, 2 * XLEN2), dtype=np.float16)
        xs[:, :XLEN] = xp_c[:, _ROWIDXA, :].reshape(128, XLEN)
        xs[:, XLEN2 : XLEN2 + XLEN] = xp_c[:, _ROWIDXB, :].reshape(128, XLEN)

        kk = kern[b, c0 * K * K : (c0 + CP) * K * K].reshape(
            CP, K * K, YG, 2, HR, W
        )

        kA_blocks, kF_blocks = [], []
        for h in (0, 1):
            for st, dstl in (("A", kA_blocks), ("F", kF_blocks)):
                for bs in _SCHED[h]["batches"][st]:
                    blks = []
                    for s in bs:
                        blk = kk[:, s, :, h].reshape(128, UF)
                        if st == "A":
                            blks.append(
                                np.clip(blk, -15.5, 15.5).astype(
                                    ml_dtypes.float8_e3m4
                                )
                            )
                        else:
                            blks.append(blk.astype(np.float16))
                    dstl.append(np.concatenate(blks, axis=1))

        in_maps.append(
            {
                "xs": np.ascontiguousarray(xs),
                "kA": np.concatenate([b_.reshape(-1) for b_ in kA_blocks]),
                "kF": np.concatenate([b_.reshape(-1) for b_ in kF_blocks]),
                "ident": _IDENT,
            }
        )
    return in_maps


def kernel(x, kernel, kernel_size):
    from concourse.bass_utils import run_bass_kernel_spmd

    in_maps = _make_in_maps(x, kernel)
    nc = _get_nc()
    res = run_bass_kernel_spmd(nc, in_maps, list(range(8)))

    out = np.empty((B, C, H, W), dtype=np.float32)
    for core in range(8):
        b, half = divmod(core, 2)
        c0 = half * CP
        o = res.results[core]["out"].reshape(CP, YG, 2, HR, W)
        out[b, c0 : c0 + CP] = o.reshape(CP, H, W).astype(np.float32)
    return out
